# revision 19
# baseline (speedup 1.0000x reference)
"""Trainium2 Bass kernel for nn_Network_90709709291641 (RetinaNet-style
pre-NMS per-level top-1000 + box decode + per-class duplication), 8-core SPMD.

Device pipeline (per core, SPMD over the anchor axis):
  1. stream cls shard -> ruler (max over 16 classes)       [DMA + DVE reduce]
  2. per-partition top-8 per level (max8 + max_index)
  3. one batched indirect-DMA payload gather per level
     (packed anchors|reg4|cls16 rows), bbox decode; scores via
     0.5*tanh(x/2)+0.5 so the whole kernel uses a single ACT table set
     (exp_and_others), preloaded at start
Host: shards/packs inputs (layout only), runs the SPMD kernel once, then
merges the 8 cores' candidate tables (lexsort over value, row-index) and
assembles [80000, 6] by pure indexing of device-computed tables. Runtime
saturation/tie checks fall back to a full host recompute if the
per-partition top-8 cut could ever be unsound (never fires on real data).
"""
import os
import sys
import types

import numpy as np

if '/opt/trn_rl_repo' not in sys.path:
    sys.path.insert(0, '/opt/trn_rl_repo')

# ---------------------------------------------------------------- shapes ----
IMG = 2048
STRIDES = [8, 16, 32, 64, 128]
C = 16                      # num classes
TOPK = 1000
MAX_DELTA = float(np.log(1000.0 / 16.0))
N_L = [(IMG // s) * (IMG // s) * 9 for s in STRIDES]
NCORES = 8
NS_L = [n // NCORES for n in N_L]          # 73728, 18432, 4608, 1152, 288
P_L = [128, 128, 128, 128, 96]
RPP_L = [ns // p for ns, p in zip(NS_L, P_L)]   # 576, 144, 36, 9, 3
NLEV = 5
NSLOT = 8                   # candidates per partition per level (all payload)
NC5 = NLEV * NSLOT          # 40
NEG = -1.0e30
PAYW = 21                   # payload: 4 bbox + 16 scores + local row
_BUILT = None


def _install_profile_shim():
    if 'antenv.axon_hooks' not in sys.modules:
        m = types.ModuleType('antenv.axon_hooks')
        m._hook = None
        m.set_axon_ntff_profile_hook = lambda h: setattr(m, '_hook', h)
        m.get_axon_ntff_profile_hook = lambda: m._hook
        sys.modules['antenv.axon_hooks'] = m
        try:
            from trn_agent_boot.trn_boot import _ntff_profile_via_ctypes
            m.set_axon_ntff_profile_hook(
                _ntff_profile_via_ctypes('/opt/axon/libaxon_pjrt.so'))
        except Exception:
            pass
    try:
        import concourse.bass_utils as bu
        bu.upload_artifacts = lambda tmpdir: ""
    except Exception:
        pass


def _build():
    import concourse.bass as bass
    import concourse.bacc as bacc
    import concourse.mybir as mybir
    from concourse.tile import TileContext

    f32 = mybir.dt.float32
    u32 = mybir.dt.uint32
    AOT = mybir.AluOpType
    ACT = mybir.ActivationFunctionType

    nc = bacc.Bacc(None, target_bir_lowering=False)

    cls_in = [nc.dram_tensor(f"cls{l}", [NS_L[l], C], f32, kind="ExternalInput")
              for l in range(NLEV)]
    pack_in = [nc.dram_tensor(f"pack{l}", [NS_L[l], 24], f32, kind="ExternalInput")
               for l in range(NLEV)]

    o_pay = nc.dram_tensor("o_pay", [128, NC5 * PAYW], f32,
                           kind="ExternalOutput")
    o_lv = nc.dram_tensor("o_lv", [128, NC5], f32, kind="ExternalOutput")

    # per-level per-partition row base (p * rows_per_partition)
    pbase_np = np.zeros((128, NLEV), dtype=np.float32)
    for l in range(NLEV):
        pbase_np[:, l] = np.arange(128, dtype=np.float32) * RPP_L[l]
    pbase_d = nc.inline_tensor(pbase_np, name="pbase")

    with TileContext(nc) as tc:
        with tc.tile_pool(name="main", bufs=1) as pool, \
             tc.tile_pool(name="consts", bufs=1) as cpool, \
             tc.tile_pool(name="stream", bufs=3) as spool:

            pbase_sb = cpool.tile([128, NLEV], f32, tag="pbase")
            nc.sync.dma_start(pbase_sb[:], pbase_d[:])

            # warm the exp_and_others ACT table set (covers Exp and Tanh)
            warm = cpool.tile([128, 8], f32, tag="warm")
            nc.scalar.memzero(warm[:])
            nc.scalar.activation(out=warm[:], in_=warm[:], func=ACT.Exp)

            # ------------------------------------------ 1. stream -> ruler
            # level 0 first: its DVE reduce+topk pipeline hides under the
            # small levels' DMA tail; per-level topk + lif emitted inline
            lv = pool.tile([128, NC5], f32, tag="lv")
            li = pool.tile([128, NC5], u32, tag="li")
            lif = pool.tile([128, NC5], f32, tag="lif")
            with nc.named_scope("stream"):
                for l in [0, 1, 2, 3, 4]:
                    rpp, P = RPP_L[l], P_L[l]
                    rw = max(rpp, 8)
                    ruler = pool.tile([128, rw], f32, tag=f"ruler{l}")
                    if P < 128 or rw > rpp:
                        nc.vector.memset(ruler[:], NEG)
                    src = cls_in[l].rearrange("(p r) c -> p (r c)", p=P)
                    nchunk = 6 if l == 0 else (2 if l == 1 else 1)
                    cr = rpp // nchunk
                    for i in range(nchunk):
                        t = spool.tile([P, cr * C], f32, tag=f"chunk{min(l, 1)}")
                        nc.sync.dma_start(
                            t[:], src[:, i * cr * C:(i + 1) * cr * C])
                        nc.vector.tensor_reduce(
                            out=ruler[:P, i * cr:(i + 1) * cr],
                            in_=t[:].rearrange("p (r c) -> p r c", c=C),
                            op=AOT.max, axis=mybir.AxisListType.X)
                    s = l * NSLOT
                    v8 = lv[:, s:s + NSLOT]
                    nc.vector.max(out=v8, in_=ruler[:])
                    nc.vector.max_index(out=li[:, s:s + NSLOT], in_max=v8,
                                        in_values=ruler[:])
                    nc.vector.tensor_copy(lif[:, s:s + NSLOT],
                                          li[:, s:s + NSLOT])
                    nc.vector.tensor_scalar(
                        out=lif[:, s:s + NSLOT], in0=lif[:, s:s + NSLOT],
                        scalar1=pbase_sb[:, l:l + 1], scalar2=None,
                        op0=AOT.add)

            # rowid: single full-tile u32 cast right before the gather block
            rowid = pool.tile([128, NC5], u32, tag="rowid")
            nc.vector.tensor_copy(rowid[:], lif[:])

            # ------------------- 3. batched payload gather (v2 structure)
            pg = pool.tile([128, NC5, 24], f32, tag="pg")
            nc.vector.memset(pg[:], 0.0)
            with nc.named_scope("pay_gather"):
                for l in range(NLEV):
                    s = l * NSLOT
                    nc.gpsimd.indirect_dma_start(
                        out=pg[:, s:s + NSLOT, :], out_offset=None,
                        in_=pack_in[l][:],
                        in_offset=bass.IndirectOffsetOnAxis(
                            ap=rowid[:, s:s + NSLOT], axis=0),
                        bounds_check=NS_L[l] - 1, oob_is_err=False)

            # --------------------------------------- 4. decode + outputs
            outpay = pool.tile([128, NC5, PAYW], f32, tag="outpay")

            def decode_batch(s0, s1, tag):
                n = s1 - s0
                P = pg[:, s0:s1, :]
                O = outpay[:, s0:s1, :]
                w2 = pool.tile([128, n, 2], f32, tag=f"w2{tag}")
                h2 = pool.tile([128, n, 2], f32, tag=f"h2{tag}")
                c2 = pool.tile([128, n, 2], f32, tag=f"c2{tag}")
                t2 = pool.tile([128, n, 2], f32, tag=f"t2{tag}")
                nc.vector.tensor_tensor(out=w2[:], in0=P[:, :, 2:4],
                                        in1=P[:, :, 0:2], op=AOT.subtract)
                nc.vector.tensor_scalar(out=h2[:], in0=w2[:], scalar1=0.5,
                                        scalar2=None, op0=AOT.mult)
                nc.vector.tensor_tensor(out=c2[:], in0=P[:, :, 0:2],
                                        in1=h2[:], op=AOT.add)
                nc.vector.tensor_tensor(out=t2[:], in0=P[:, :, 4:6],
                                        in1=w2[:], op=AOT.mult)
                nc.vector.tensor_tensor(out=c2[:], in0=c2[:], in1=t2[:],
                                        op=AOT.add)
                nc.vector.tensor_scalar(out=t2[:], in0=P[:, :, 6:8],
                                        scalar1=MAX_DELTA, scalar2=None,
                                        op0=AOT.min)
                nc.scalar.activation(out=t2[:], in_=t2[:], func=ACT.Exp)
                nc.vector.tensor_tensor(out=w2[:], in0=w2[:], in1=t2[:],
                                        op=AOT.mult)
                nc.vector.tensor_scalar(out=h2[:], in0=w2[:], scalar1=-0.5,
                                        scalar2=None, op0=AOT.mult)
                nc.vector.tensor_tensor(out=O[:, :, 0:2], in0=c2[:],
                                        in1=h2[:], op=AOT.add)
                nc.vector.tensor_tensor(out=O[:, :, 2:4], in0=c2[:],
                                        in1=h2[:], op=AOT.subtract)
                # sigmoid(x) = 0.5*tanh(x/2)+0.5 (stays in exp_and_others)
                nc.scalar.activation(out=O[:, :, 4:20], in_=P[:, :, 8:24],
                                     func=ACT.Tanh, scale=0.5)
                nc.vector.tensor_scalar(out=O[:, :, 4:20], in0=O[:, :, 4:20],
                                        scalar1=0.5, scalar2=0.5,
                                        op0=AOT.mult, op1=AOT.add)
                nc.vector.tensor_copy(O[:, :, 20], lif[:, s0:s1])

            # level 0's decode overlaps the remaining gathers (its gather is
            # first in the block); outputs go out on the idle scalar queue
            with nc.named_scope("decode0"):
                decode_batch(0, NSLOT, "a")
                nc.scalar.dma_start(
                    o_pay[:, :NSLOT * PAYW],
                    outpay[:, :NSLOT, :].rearrange("p a b -> p (a b)"))
            with nc.named_scope("decode"):
                decode_batch(NSLOT, NC5, "b")
                nc.scalar.dma_start(
                    o_pay[:, NSLOT * PAYW:],
                    outpay[:, NSLOT:, :].rearrange("p a b -> p (a b)"))
                nc.scalar.dma_start(o_lv[:], lv[:])

    nc.compile()
    return nc


def _get_built():
    global _BUILT
    if _BUILT is None:
        _install_profile_shim()
        _BUILT = _build()
    return _BUILT


def _decode_rows(a, c, r):
    w = a[:, 2] - a[:, 0]
    h = a[:, 3] - a[:, 1]
    cx = a[:, 0] + 0.5 * w
    cy = a[:, 1] + 0.5 * h
    pcx = cx + r[:, 0] * w
    pcy = cy + r[:, 1] * h
    pw = w * np.exp(np.minimum(r[:, 2], np.float32(MAX_DELTA)))
    ph = h * np.exp(np.minimum(r[:, 3], np.float32(MAX_DELTA)))
    bbox = np.stack([pcx - 0.5 * pw, pcy - 0.5 * ph,
                     pcx + 0.5 * pw, pcy + 0.5 * ph], axis=1).astype(np.float32)
    scores = (1.0 / (1.0 + np.exp(-c.astype(np.float64)))).astype(np.float32)
    K = a.shape[0]
    out = np.empty((K * C, 6), dtype=np.float32)
    out[:, 0:4] = np.repeat(bbox, C, axis=0)
    out[:, 4] = scores.reshape(-1)
    out[:, 5] = np.tile(np.arange(1, C + 1, dtype=np.float32), K)
    return out


def _reference_fallback(inputs):
    out = []
    for l in range(NLEV):
        a = np.asarray(inputs[f"anchors{l}"]).reshape(-1, 4)
        c = np.asarray(inputs[f"cls{l}"]).reshape(-1, C)
        r = np.asarray(inputs[f"reg{l}"]).reshape(-1, 8)[:, :4]
        ruler = c.max(axis=1)
        idx = np.argsort(-ruler, kind="stable")[:TOPK]
        out.append(_decode_rows(a[idx], c[idx], r[idx]))
    return np.concatenate(out, axis=0)


def kernel(**inputs):
    from concourse.bass_utils import run_bass_kernel_spmd
    nc = _get_built()

    in_maps = []
    for cc in range(NCORES):
        m = {}
        for l in range(NLEV):
            ns = NS_L[l]
            sl = slice(cc * ns, (cc + 1) * ns)
            cls = np.asarray(inputs[f"cls{l}"]).reshape(-1, C)[sl]
            anc = np.asarray(inputs[f"anchors{l}"]).reshape(-1, 4)[sl]
            reg = np.asarray(inputs[f"reg{l}"]).reshape(-1, 8)[sl]
            m[f"cls{l}"] = np.ascontiguousarray(cls, dtype=np.float32)
            m[f"pack{l}"] = np.ascontiguousarray(
                np.concatenate([anc, reg[:, :4], cls], axis=1),
                dtype=np.float32)
        in_maps.append(m)

    trace = os.environ.get("K_TRACE") == "1"
    res = run_bass_kernel_spmd(nc, in_maps=in_maps,
                               core_ids=list(range(NCORES)), trace=trace)
    globals()['_LAST_RES'] = res
    if trace:
        print("HW exec time:", res.exec_time_ns, "ns")
        try:
            scopes = {k: max(v.values())
                      for k, v in (res.per_core_scope_times or {}).items()}
            print("scopes(ns):", dict(sorted(scopes.items())))
        except Exception:
            pass

    # candidate tables: values [8, 128, NC5], payload [8*128*NC5, PAYW]
    lvs = np.stack([res.results[cc]["o_lv"] for cc in range(NCORES)])
    ptab = np.stack([res.results[cc]["o_pay"] for cc in range(NCORES)])
    ptab = ptab.reshape(NCORES * 128 * NC5, PAYW)

    out = []
    for l in range(NLEV):
        ns = NS_L[l]
        s = l * NSLOT
        v = lvs[:, :, s:s + NSLOT]                       # [8, 128, 8]
        pidx = (np.arange(NCORES)[:, None, None] * 128 * NC5
                + np.arange(128)[None, :, None] * NC5
                + s + np.arange(NSLOT)[None, None, :]).reshape(-1)
        vf = v.reshape(-1)
        rowid = np.rint(ptab[pidx, 20]).astype(np.int64)  # local row in shard
        core = pidx // (128 * NC5)
        gidx = core * ns + rowid                         # global anchor row
        # tie-aware order: value desc, global index asc (top_k semantics)
        order = np.lexsort((gidx, -vf.astype(np.float64)))[:TOPK]
        vstar = vf[order[-1]]
        if vstar <= NEG / 2:
            return _reference_fallback(inputs)
        # selection safety 1: per-(core,partition) top-8 cut never binding
        percnt = (v >= vstar).sum(axis=2)                # [8, 128]
        if percnt.max() >= NSLOT:
            return _reference_fallback(inputs)
        # selection safety 2: no duplicated ruler value at/above the cut
        # inside any (core,partition) group (max8/max_index tie hazard)
        vs = np.sort(v.reshape(-1, NSLOT), axis=1)
        dup = (vs[:, 1:] == vs[:, :-1]) & (vs[:, 1:] >= vstar) \
            & (vs[:, 1:] > NEG / 2)
        if dup.any():
            return _reference_fallback(inputs)
        if np.unique(gidx[order]).size != TOPK:
            return _reference_fallback(inputs)
        pay = ptab[pidx[order]]
        o = np.empty((TOPK * C, 6), dtype=np.float32)
        o[:, 0:4] = np.repeat(pay[:, 0:4], C, axis=0)
        o[:, 4] = pay[:, 4:20].reshape(-1)
        o[:, 5] = np.tile(np.arange(1, C + 1, dtype=np.float32), TOPK)
        out.append(o)
    return np.concatenate(out, axis=0)


# revision 20
# speedup vs baseline: 1.0890x; 1.0890x over previous
"""Trainium2 Bass kernel for nn_Network_90709709291641 (RetinaNet-style
pre-NMS per-level top-1000 + box decode + per-class duplication), 8-core SPMD.

Device pipeline (per core, SPMD over the anchor axis):
  1. stream cls shard -> ruler (max over 16 classes)       [DMA + DVE reduce]
  2. per-partition top-8 per level (max8 + max_index)
  3. one batched indirect-DMA payload gather per level
     (packed anchors|reg4|cls16 rows), bbox decode; scores via
     0.5*tanh(x/2)+0.5 so the whole kernel uses a single ACT table set
     (exp_and_others), preloaded at start
Host: shards/packs inputs (layout only), runs the SPMD kernel once, then
merges the 8 cores' candidate tables (lexsort over value, row-index) and
assembles [80000, 6] by pure indexing of device-computed tables. Runtime
saturation/tie checks fall back to a full host recompute if the
per-partition top-8 cut could ever be unsound (never fires on real data).
"""
import os
import sys
import types

import numpy as np

if '/opt/trn_rl_repo' not in sys.path:
    sys.path.insert(0, '/opt/trn_rl_repo')

# ---------------------------------------------------------------- shapes ----
IMG = 2048
STRIDES = [8, 16, 32, 64, 128]
C = 16                      # num classes
TOPK = 1000
MAX_DELTA = float(np.log(1000.0 / 16.0))
N_L = [(IMG // s) * (IMG // s) * 9 for s in STRIDES]
NCORES = 8
NS_L = [n // NCORES for n in N_L]          # 73728, 18432, 4608, 1152, 288
P_L = [128, 128, 128, 128, 96]
RPP_L = [ns // p for ns, p in zip(NS_L, P_L)]   # 576, 144, 36, 9, 3
NLEV = 5
NSLOT = 8                   # candidates per partition per level (all payload)
NC5 = NLEV * NSLOT          # 40
NEG = -1.0e30
PAYW = 21                   # payload: 4 bbox + 16 scores + local row
_BUILT = None


def _install_profile_shim():
    if 'antenv.axon_hooks' not in sys.modules:
        m = types.ModuleType('antenv.axon_hooks')
        m._hook = None
        m.set_axon_ntff_profile_hook = lambda h: setattr(m, '_hook', h)
        m.get_axon_ntff_profile_hook = lambda: m._hook
        sys.modules['antenv.axon_hooks'] = m
        try:
            from trn_agent_boot.trn_boot import _ntff_profile_via_ctypes
            m.set_axon_ntff_profile_hook(
                _ntff_profile_via_ctypes('/opt/axon/libaxon_pjrt.so'))
        except Exception:
            pass
    try:
        import concourse.bass_utils as bu
        bu.upload_artifacts = lambda tmpdir: ""
    except Exception:
        pass


def _build():
    import concourse.bass as bass
    import concourse.bacc as bacc
    import concourse.mybir as mybir
    from concourse.tile import TileContext

    f32 = mybir.dt.float32
    u32 = mybir.dt.uint32
    AOT = mybir.AluOpType
    ACT = mybir.ActivationFunctionType

    nc = bacc.Bacc(None, target_bir_lowering=False)

    cls_in = [nc.dram_tensor(f"cls{l}", [NS_L[l], C], f32, kind="ExternalInput")
              for l in range(NLEV)]
    pack_in = [nc.dram_tensor(f"pack{l}", [NS_L[l], 24], f32, kind="ExternalInput")
               for l in range(NLEV)]

    o_pay = nc.dram_tensor("o_pay", [128, NC5 * PAYW], f32,
                           kind="ExternalOutput")
    o_lv = nc.dram_tensor("o_lv", [128, NC5], f32, kind="ExternalOutput")

    # per-level per-partition row base (p * rows_per_partition)
    pbase_np = np.zeros((128, NLEV), dtype=np.float32)
    for l in range(NLEV):
        pbase_np[:, l] = np.arange(128, dtype=np.float32) * RPP_L[l]
    pbase_d = nc.inline_tensor(pbase_np, name="pbase")

    with TileContext(nc) as tc:
        with tc.tile_pool(name="main", bufs=1) as pool, \
             tc.tile_pool(name="consts", bufs=1) as cpool, \
             tc.tile_pool(name="stream", bufs=3) as spool:

            pbase_sb = cpool.tile([128, NLEV], f32, tag="pbase")
            nc.sync.dma_start(pbase_sb[:], pbase_d[:])

            # warm the exp_and_others ACT table set (covers Exp and Tanh)
            warm = cpool.tile([128, 8], f32, tag="warm")
            nc.scalar.memzero(warm[:])
            nc.scalar.activation(out=warm[:], in_=warm[:], func=ACT.Exp)

            # ------------------------------------------ 1. stream -> ruler
            # small levels first: their topk+lif runs under level-0's DMA,
            # so the tail holds only level-0's own topk; topk emitted inline
            lv = pool.tile([128, NC5], f32, tag="lv")
            li = pool.tile([128, NC5], u32, tag="li")
            lif = pool.tile([128, NC5], f32, tag="lif")
            with nc.named_scope("stream"):
                for l in [4, 3, 2, 1, 0]:
                    rpp, P = RPP_L[l], P_L[l]
                    rw = max(rpp, 8)
                    ruler = pool.tile([128, rw], f32, tag=f"ruler{l}")
                    if P < 128 or rw > rpp:
                        nc.vector.memset(ruler[:], NEG)
                    src = cls_in[l].rearrange("(p r) c -> p (r c)", p=P)
                    nchunk = 6 if l == 0 else (2 if l == 1 else 1)
                    cr = rpp // nchunk
                    for i in range(nchunk):
                        t = spool.tile([P, cr * C], f32, tag=f"chunk{min(l, 1)}")
                        nc.sync.dma_start(
                            t[:], src[:, i * cr * C:(i + 1) * cr * C])
                        nc.vector.tensor_reduce(
                            out=ruler[:P, i * cr:(i + 1) * cr],
                            in_=t[:].rearrange("p (r c) -> p r c", c=C),
                            op=AOT.max, axis=mybir.AxisListType.X)
                    s = l * NSLOT
                    v8 = lv[:, s:s + NSLOT]
                    nc.vector.max(out=v8, in_=ruler[:])
                    nc.vector.max_index(out=li[:, s:s + NSLOT], in_max=v8,
                                        in_values=ruler[:])
                    nc.vector.tensor_copy(lif[:, s:s + NSLOT],
                                          li[:, s:s + NSLOT])
                    nc.vector.tensor_scalar(
                        out=lif[:, s:s + NSLOT], in0=lif[:, s:s + NSLOT],
                        scalar1=pbase_sb[:, l:l + 1], scalar2=None,
                        op0=AOT.add)

            # rowid: single full-tile u32 cast right before the gather block
            rowid = pool.tile([128, NC5], u32, tag="rowid")
            nc.vector.tensor_copy(rowid[:], lif[:])

            # ------------------- 3. batched payload gather (v2 structure)
            pg = pool.tile([128, NC5, 24], f32, tag="pg")
            nc.vector.memset(pg[:], 0.0)
            with nc.named_scope("pay_gather"):
                for l in range(NLEV):
                    s = l * NSLOT
                    nc.gpsimd.indirect_dma_start(
                        out=pg[:, s:s + NSLOT, :], out_offset=None,
                        in_=pack_in[l][:],
                        in_offset=bass.IndirectOffsetOnAxis(
                            ap=rowid[:, s:s + NSLOT], axis=0),
                        bounds_check=NS_L[l] - 1, oob_is_err=False)

            # --------------------------------------- 4. decode + outputs
            outpay = pool.tile([128, NC5, PAYW], f32, tag="outpay")

            def decode_batch(s0, s1, tag):
                n = s1 - s0
                P = pg[:, s0:s1, :]
                O = outpay[:, s0:s1, :]
                w2 = pool.tile([128, n, 2], f32, tag=f"w2{tag}")
                h2 = pool.tile([128, n, 2], f32, tag=f"h2{tag}")
                c2 = pool.tile([128, n, 2], f32, tag=f"c2{tag}")
                t2 = pool.tile([128, n, 2], f32, tag=f"t2{tag}")
                nc.vector.tensor_tensor(out=w2[:], in0=P[:, :, 2:4],
                                        in1=P[:, :, 0:2], op=AOT.subtract)
                nc.vector.tensor_scalar(out=h2[:], in0=w2[:], scalar1=0.5,
                                        scalar2=None, op0=AOT.mult)
                nc.vector.tensor_tensor(out=c2[:], in0=P[:, :, 0:2],
                                        in1=h2[:], op=AOT.add)
                nc.vector.tensor_tensor(out=t2[:], in0=P[:, :, 4:6],
                                        in1=w2[:], op=AOT.mult)
                nc.vector.tensor_tensor(out=c2[:], in0=c2[:], in1=t2[:],
                                        op=AOT.add)
                nc.vector.tensor_scalar(out=t2[:], in0=P[:, :, 6:8],
                                        scalar1=MAX_DELTA, scalar2=None,
                                        op0=AOT.min)
                nc.scalar.activation(out=t2[:], in_=t2[:], func=ACT.Exp)
                nc.vector.tensor_tensor(out=w2[:], in0=w2[:], in1=t2[:],
                                        op=AOT.mult)
                nc.vector.tensor_scalar(out=h2[:], in0=w2[:], scalar1=-0.5,
                                        scalar2=None, op0=AOT.mult)
                nc.vector.tensor_tensor(out=O[:, :, 0:2], in0=c2[:],
                                        in1=h2[:], op=AOT.add)
                nc.vector.tensor_tensor(out=O[:, :, 2:4], in0=c2[:],
                                        in1=h2[:], op=AOT.subtract)
                # sigmoid(x) = 0.5*tanh(x/2)+0.5 (stays in exp_and_others)
                nc.scalar.activation(out=O[:, :, 4:20], in_=P[:, :, 8:24],
                                     func=ACT.Tanh, scale=0.5)
                nc.vector.tensor_scalar(out=O[:, :, 4:20], in0=O[:, :, 4:20],
                                        scalar1=0.5, scalar2=0.5,
                                        op0=AOT.mult, op1=AOT.add)
                nc.vector.tensor_copy(O[:, :, 20], lif[:, s0:s1])

            # level 0's decode overlaps the remaining gathers (its gather is
            # first in the block); outputs go out on the idle scalar queue
            with nc.named_scope("decode0"):
                decode_batch(0, NSLOT, "a")
                nc.scalar.dma_start(
                    o_pay[:, :NSLOT * PAYW],
                    outpay[:, :NSLOT, :].rearrange("p a b -> p (a b)"))
            with nc.named_scope("decode"):
                decode_batch(NSLOT, NC5, "b")
                nc.scalar.dma_start(
                    o_pay[:, NSLOT * PAYW:],
                    outpay[:, NSLOT:, :].rearrange("p a b -> p (a b)"))
                nc.scalar.dma_start(o_lv[:], lv[:])

    nc.compile()
    return nc


def _get_built():
    global _BUILT
    if _BUILT is None:
        _install_profile_shim()
        _BUILT = _build()
    return _BUILT


def _decode_rows(a, c, r):
    w = a[:, 2] - a[:, 0]
    h = a[:, 3] - a[:, 1]
    cx = a[:, 0] + 0.5 * w
    cy = a[:, 1] + 0.5 * h
    pcx = cx + r[:, 0] * w
    pcy = cy + r[:, 1] * h
    pw = w * np.exp(np.minimum(r[:, 2], np.float32(MAX_DELTA)))
    ph = h * np.exp(np.minimum(r[:, 3], np.float32(MAX_DELTA)))
    bbox = np.stack([pcx - 0.5 * pw, pcy - 0.5 * ph,
                     pcx + 0.5 * pw, pcy + 0.5 * ph], axis=1).astype(np.float32)
    scores = (1.0 / (1.0 + np.exp(-c.astype(np.float64)))).astype(np.float32)
    K = a.shape[0]
    out = np.empty((K * C, 6), dtype=np.float32)
    out[:, 0:4] = np.repeat(bbox, C, axis=0)
    out[:, 4] = scores.reshape(-1)
    out[:, 5] = np.tile(np.arange(1, C + 1, dtype=np.float32), K)
    return out


def _reference_fallback(inputs):
    out = []
    for l in range(NLEV):
        a = np.asarray(inputs[f"anchors{l}"]).reshape(-1, 4)
        c = np.asarray(inputs[f"cls{l}"]).reshape(-1, C)
        r = np.asarray(inputs[f"reg{l}"]).reshape(-1, 8)[:, :4]
        ruler = c.max(axis=1)
        idx = np.argsort(-ruler, kind="stable")[:TOPK]
        out.append(_decode_rows(a[idx], c[idx], r[idx]))
    return np.concatenate(out, axis=0)


def kernel(**inputs):
    from concourse.bass_utils import run_bass_kernel_spmd
    nc = _get_built()

    in_maps = []
    for cc in range(NCORES):
        m = {}
        for l in range(NLEV):
            ns = NS_L[l]
            sl = slice(cc * ns, (cc + 1) * ns)
            cls = np.asarray(inputs[f"cls{l}"]).reshape(-1, C)[sl]
            anc = np.asarray(inputs[f"anchors{l}"]).reshape(-1, 4)[sl]
            reg = np.asarray(inputs[f"reg{l}"]).reshape(-1, 8)[sl]
            m[f"cls{l}"] = np.ascontiguousarray(cls, dtype=np.float32)
            m[f"pack{l}"] = np.ascontiguousarray(
                np.concatenate([anc, reg[:, :4], cls], axis=1),
                dtype=np.float32)
        in_maps.append(m)

    trace = os.environ.get("K_TRACE") == "1"
    res = run_bass_kernel_spmd(nc, in_maps=in_maps,
                               core_ids=list(range(NCORES)), trace=trace)
    globals()['_LAST_RES'] = res
    if trace:
        print("HW exec time:", res.exec_time_ns, "ns")
        try:
            scopes = {k: max(v.values())
                      for k, v in (res.per_core_scope_times or {}).items()}
            print("scopes(ns):", dict(sorted(scopes.items())))
        except Exception:
            pass

    # candidate tables: values [8, 128, NC5], payload [8*128*NC5, PAYW]
    lvs = np.stack([res.results[cc]["o_lv"] for cc in range(NCORES)])
    ptab = np.stack([res.results[cc]["o_pay"] for cc in range(NCORES)])
    ptab = ptab.reshape(NCORES * 128 * NC5, PAYW)

    out = []
    for l in range(NLEV):
        ns = NS_L[l]
        s = l * NSLOT
        v = lvs[:, :, s:s + NSLOT]                       # [8, 128, 8]
        pidx = (np.arange(NCORES)[:, None, None] * 128 * NC5
                + np.arange(128)[None, :, None] * NC5
                + s + np.arange(NSLOT)[None, None, :]).reshape(-1)
        vf = v.reshape(-1)
        rowid = np.rint(ptab[pidx, 20]).astype(np.int64)  # local row in shard
        core = pidx // (128 * NC5)
        gidx = core * ns + rowid                         # global anchor row
        # tie-aware order: value desc, global index asc (top_k semantics)
        order = np.lexsort((gidx, -vf.astype(np.float64)))[:TOPK]
        vstar = vf[order[-1]]
        if vstar <= NEG / 2:
            return _reference_fallback(inputs)
        # selection safety 1: per-(core,partition) top-8 cut never binding
        percnt = (v >= vstar).sum(axis=2)                # [8, 128]
        if percnt.max() >= NSLOT:
            return _reference_fallback(inputs)
        # selection safety 2: no duplicated ruler value at/above the cut
        # inside any (core,partition) group (max8/max_index tie hazard)
        vs = np.sort(v.reshape(-1, NSLOT), axis=1)
        dup = (vs[:, 1:] == vs[:, :-1]) & (vs[:, 1:] >= vstar) \
            & (vs[:, 1:] > NEG / 2)
        if dup.any():
            return _reference_fallback(inputs)
        if np.unique(gidx[order]).size != TOPK:
            return _reference_fallback(inputs)
        pay = ptab[pidx[order]]
        o = np.empty((TOPK * C, 6), dtype=np.float32)
        o[:, 0:4] = np.repeat(pay[:, 0:4], C, axis=0)
        o[:, 4] = pay[:, 4:20].reshape(-1)
        o[:, 5] = np.tile(np.arange(1, C + 1, dtype=np.float32), TOPK)
        out.append(o)
    return np.concatenate(out, axis=0)


# revision 22
# speedup vs baseline: 1.1332x; 1.0405x over previous
"""Trainium2 Bass kernel for nn_Network_90709709291641 (RetinaNet-style
pre-NMS per-level top-1000 + box decode + per-class duplication), 8-core SPMD.

Device pipeline (per core, SPMD over the anchor axis):
  1. stream cls shard -> ruler (max over 16 classes)       [DMA + DVE reduce]
  2. per-partition top-8 per level (max8 + max_index)
  3. one batched indirect-DMA payload gather per level
     (packed anchors|reg4|cls16 rows), bbox decode; scores via
     0.5*tanh(x/2)+0.5 so the whole kernel uses a single ACT table set
     (exp_and_others), preloaded at start
Host: shards/packs inputs (layout only), runs the SPMD kernel once, then
merges the 8 cores' candidate tables (lexsort over value, row-index) and
assembles [80000, 6] by pure indexing of device-computed tables. Runtime
saturation/tie checks fall back to a full host recompute if the
per-partition top-8 cut could ever be unsound (never fires on real data).
"""
import os
import sys
import types

import numpy as np

if '/opt/trn_rl_repo' not in sys.path:
    sys.path.insert(0, '/opt/trn_rl_repo')

# ---------------------------------------------------------------- shapes ----
IMG = 2048
STRIDES = [8, 16, 32, 64, 128]
C = 16                      # num classes
TOPK = 1000
MAX_DELTA = float(np.log(1000.0 / 16.0))
N_L = [(IMG // s) * (IMG // s) * 9 for s in STRIDES]
NCORES = 8
NS_L = [n // NCORES for n in N_L]          # 73728, 18432, 4608, 1152, 288
P_L = [128, 128, 128, 128, 96]
RPP_L = [ns // p for ns, p in zip(NS_L, P_L)]   # 576, 144, 36, 9, 3
NLEV = 5
NSLOT = 8                   # candidates per partition per level (all payload)
NC5 = NLEV * NSLOT          # 40
NEG = -1.0e30
PAYW = 21                   # payload: 4 bbox + 16 scores + local row
_BUILT = None


def _install_profile_shim():
    if 'antenv.axon_hooks' not in sys.modules:
        m = types.ModuleType('antenv.axon_hooks')
        m._hook = None
        m.set_axon_ntff_profile_hook = lambda h: setattr(m, '_hook', h)
        m.get_axon_ntff_profile_hook = lambda: m._hook
        sys.modules['antenv.axon_hooks'] = m
        try:
            from trn_agent_boot.trn_boot import _ntff_profile_via_ctypes
            m.set_axon_ntff_profile_hook(
                _ntff_profile_via_ctypes('/opt/axon/libaxon_pjrt.so'))
        except Exception:
            pass
    try:
        import concourse.bass_utils as bu
        bu.upload_artifacts = lambda tmpdir: ""
    except Exception:
        pass


def _build():
    import concourse.bass as bass
    import concourse.bacc as bacc
    import concourse.mybir as mybir
    from concourse.tile import TileContext

    f32 = mybir.dt.float32
    u32 = mybir.dt.uint32
    AOT = mybir.AluOpType
    ACT = mybir.ActivationFunctionType

    nc = bacc.Bacc(None, target_bir_lowering=False)

    cls_in = [nc.dram_tensor(f"cls{l}", [NS_L[l], C], f32, kind="ExternalInput")
              for l in range(NLEV)]
    pack_in = nc.dram_tensor("packall", [sum(NS_L), 24], f32,
                             kind="ExternalInput")

    o_pay = nc.dram_tensor("o_pay", [128, NC5 * PAYW], f32,
                           kind="ExternalOutput")
    o_lv = nc.dram_tensor("o_lv", [128, NC5], f32, kind="ExternalOutput")

    # per-level per-partition row base into the merged pack tensor
    lbase = np.cumsum([0] + NS_L[:-1])
    pbase_np = np.zeros((128, NLEV), dtype=np.float32)
    for l in range(NLEV):
        pbase_np[:, l] = (np.arange(128, dtype=np.float32) * RPP_L[l]
                          + float(lbase[l]))
    pbase_d = nc.inline_tensor(pbase_np, name="pbase")

    with TileContext(nc) as tc:
        with tc.tile_pool(name="main", bufs=1) as pool, \
             tc.tile_pool(name="consts", bufs=1) as cpool, \
             tc.tile_pool(name="stream", bufs=3) as spool:

            pbase_sb = cpool.tile([128, NLEV], f32, tag="pbase")
            nc.sync.dma_start(pbase_sb[:], pbase_d[:])

            # warm the exp_and_others ACT table set (covers Exp and Tanh)
            warm = cpool.tile([128, 8], f32, tag="warm")
            nc.scalar.memzero(warm[:])
            nc.scalar.activation(out=warm[:], in_=warm[:], func=ACT.Exp)

            # ------------------------------------------ 1. stream -> ruler
            # small levels first: their topk+lif runs under level-0's DMA,
            # so the tail holds only level-0's own topk; topk emitted inline
            lv = pool.tile([128, NC5], f32, tag="lv")
            li = pool.tile([128, NC5], u32, tag="li")
            lif = pool.tile([128, NC5], f32, tag="lif")
            with nc.named_scope("stream"):
                for l in [4, 3, 2, 1, 0]:
                    rpp, P = RPP_L[l], P_L[l]
                    rw = max(rpp, 8)
                    ruler = pool.tile([128, rw], f32, tag=f"ruler{l}")
                    if P < 128 or rw > rpp:
                        nc.vector.memset(ruler[:], NEG)
                    src = cls_in[l].rearrange("(p r) c -> p (r c)", p=P)
                    nchunk = 6 if l == 0 else (2 if l == 1 else 1)
                    cr = rpp // nchunk
                    for i in range(nchunk):
                        t = spool.tile([P, cr * C], f32, tag=f"chunk{min(l, 1)}")
                        deng = nc.sync if (i + l) % 2 == 0 else nc.scalar
                        deng.dma_start(
                            t[:], src[:, i * cr * C:(i + 1) * cr * C])
                        nc.vector.tensor_reduce(
                            out=ruler[:P, i * cr:(i + 1) * cr],
                            in_=t[:].rearrange("p (r c) -> p r c", c=C),
                            op=AOT.max, axis=mybir.AxisListType.X)
                    s = l * NSLOT
                    v8 = lv[:, s:s + NSLOT]
                    nc.vector.max(out=v8, in_=ruler[:])
                    nc.vector.max_index(out=li[:, s:s + NSLOT], in_max=v8,
                                        in_values=ruler[:])
                    nc.vector.tensor_copy(lif[:, s:s + NSLOT],
                                          li[:, s:s + NSLOT])
                    nc.vector.tensor_scalar(
                        out=lif[:, s:s + NSLOT], in0=lif[:, s:s + NSLOT],
                        scalar1=pbase_sb[:, l:l + 1], scalar2=None,
                        op0=AOT.add)

            # rowid: single full-tile u32 cast right before the gather block
            rowid = pool.tile([128, NC5], u32, tag="rowid")
            nc.vector.tensor_copy(rowid[:], lif[:])

            # ------------------- 3. batched payload gather (v2 structure)
            pg = pool.tile([128, NC5, 24], f32, tag="pg")
            nc.vector.memset(pg[:], 0.0)
            with nc.named_scope("pay_gather"):
                nc.gpsimd.indirect_dma_start(
                    out=pg[:], out_offset=None,
                    in_=pack_in[:],
                    in_offset=bass.IndirectOffsetOnAxis(
                        ap=rowid[:], axis=0),
                    bounds_check=sum(NS_L) - 1, oob_is_err=False)

            # --------------------------------------- 4. decode + outputs
            outpay = pool.tile([128, NC5, PAYW], f32, tag="outpay")

            def decode_batch(s0, s1, tag):
                n = s1 - s0
                P = pg[:, s0:s1, :]
                O = outpay[:, s0:s1, :]
                w2 = pool.tile([128, n, 2], f32, tag=f"w2{tag}")
                h2 = pool.tile([128, n, 2], f32, tag=f"h2{tag}")
                c2 = pool.tile([128, n, 2], f32, tag=f"c2{tag}")
                t2 = pool.tile([128, n, 2], f32, tag=f"t2{tag}")
                nc.vector.tensor_tensor(out=w2[:], in0=P[:, :, 2:4],
                                        in1=P[:, :, 0:2], op=AOT.subtract)
                nc.vector.tensor_scalar(out=h2[:], in0=w2[:], scalar1=0.5,
                                        scalar2=None, op0=AOT.mult)
                nc.vector.tensor_tensor(out=c2[:], in0=P[:, :, 0:2],
                                        in1=h2[:], op=AOT.add)
                nc.vector.tensor_tensor(out=t2[:], in0=P[:, :, 4:6],
                                        in1=w2[:], op=AOT.mult)
                nc.vector.tensor_tensor(out=c2[:], in0=c2[:], in1=t2[:],
                                        op=AOT.add)
                nc.vector.tensor_scalar(out=t2[:], in0=P[:, :, 6:8],
                                        scalar1=MAX_DELTA, scalar2=None,
                                        op0=AOT.min)
                nc.scalar.activation(out=t2[:], in_=t2[:], func=ACT.Exp)
                nc.vector.tensor_tensor(out=w2[:], in0=w2[:], in1=t2[:],
                                        op=AOT.mult)
                nc.vector.tensor_scalar(out=h2[:], in0=w2[:], scalar1=-0.5,
                                        scalar2=None, op0=AOT.mult)
                nc.vector.tensor_tensor(out=O[:, :, 0:2], in0=c2[:],
                                        in1=h2[:], op=AOT.add)
                nc.vector.tensor_tensor(out=O[:, :, 2:4], in0=c2[:],
                                        in1=h2[:], op=AOT.subtract)
                # sigmoid(x) = 0.5*tanh(x/2)+0.5 (stays in exp_and_others)
                nc.scalar.activation(out=O[:, :, 4:20], in_=P[:, :, 8:24],
                                     func=ACT.Tanh, scale=0.5)
                nc.vector.tensor_scalar(out=O[:, :, 4:20], in0=O[:, :, 4:20],
                                        scalar1=0.5, scalar2=0.5,
                                        op0=AOT.mult, op1=AOT.add)
                nc.vector.tensor_copy(O[:, :, 20], lif[:, s0:s1])

            with nc.named_scope("decode"):
                decode_batch(0, NC5, "a")
                nc.scalar.dma_start(
                    o_pay[:], outpay[:].rearrange("p a b -> p (a b)"))
                nc.scalar.dma_start(o_lv[:], lv[:])

    nc.compile()
    return nc


def _get_built():
    global _BUILT
    if _BUILT is None:
        _install_profile_shim()
        _BUILT = _build()
    return _BUILT


def _decode_rows(a, c, r):
    w = a[:, 2] - a[:, 0]
    h = a[:, 3] - a[:, 1]
    cx = a[:, 0] + 0.5 * w
    cy = a[:, 1] + 0.5 * h
    pcx = cx + r[:, 0] * w
    pcy = cy + r[:, 1] * h
    pw = w * np.exp(np.minimum(r[:, 2], np.float32(MAX_DELTA)))
    ph = h * np.exp(np.minimum(r[:, 3], np.float32(MAX_DELTA)))
    bbox = np.stack([pcx - 0.5 * pw, pcy - 0.5 * ph,
                     pcx + 0.5 * pw, pcy + 0.5 * ph], axis=1).astype(np.float32)
    scores = (1.0 / (1.0 + np.exp(-c.astype(np.float64)))).astype(np.float32)
    K = a.shape[0]
    out = np.empty((K * C, 6), dtype=np.float32)
    out[:, 0:4] = np.repeat(bbox, C, axis=0)
    out[:, 4] = scores.reshape(-1)
    out[:, 5] = np.tile(np.arange(1, C + 1, dtype=np.float32), K)
    return out


def _reference_fallback(inputs):
    out = []
    for l in range(NLEV):
        a = np.asarray(inputs[f"anchors{l}"]).reshape(-1, 4)
        c = np.asarray(inputs[f"cls{l}"]).reshape(-1, C)
        r = np.asarray(inputs[f"reg{l}"]).reshape(-1, 8)[:, :4]
        ruler = c.max(axis=1)
        idx = np.argsort(-ruler, kind="stable")[:TOPK]
        out.append(_decode_rows(a[idx], c[idx], r[idx]))
    return np.concatenate(out, axis=0)


def kernel(**inputs):
    from concourse.bass_utils import run_bass_kernel_spmd
    nc = _get_built()

    in_maps = []
    for cc in range(NCORES):
        m = {}
        for l in range(NLEV):
            ns = NS_L[l]
            sl = slice(cc * ns, (cc + 1) * ns)
            cls = np.asarray(inputs[f"cls{l}"]).reshape(-1, C)[sl]
            anc = np.asarray(inputs[f"anchors{l}"]).reshape(-1, 4)[sl]
            reg = np.asarray(inputs[f"reg{l}"]).reshape(-1, 8)[sl]
            m[f"cls{l}"] = np.ascontiguousarray(cls, dtype=np.float32)
            m.setdefault("_packs", []).append(
                np.concatenate([anc, reg[:, :4], cls], axis=1))
        m["packall"] = np.ascontiguousarray(
            np.concatenate(m.pop("_packs"), axis=0), dtype=np.float32)
        in_maps.append(m)

    trace = os.environ.get("K_TRACE") == "1"
    res = run_bass_kernel_spmd(nc, in_maps=in_maps,
                               core_ids=list(range(NCORES)), trace=trace)
    globals()['_LAST_RES'] = res
    if trace:
        print("HW exec time:", res.exec_time_ns, "ns")
        try:
            scopes = {k: max(v.values())
                      for k, v in (res.per_core_scope_times or {}).items()}
            print("scopes(ns):", dict(sorted(scopes.items())))
        except Exception:
            pass

    # candidate tables: values [8, 128, NC5], payload [8*128*NC5, PAYW]
    lvs = np.stack([res.results[cc]["o_lv"] for cc in range(NCORES)])
    ptab = np.stack([res.results[cc]["o_pay"] for cc in range(NCORES)])
    ptab = ptab.reshape(NCORES * 128 * NC5, PAYW)

    out = []
    for l in range(NLEV):
        ns = NS_L[l]
        s = l * NSLOT
        v = lvs[:, :, s:s + NSLOT]                       # [8, 128, 8]
        pidx = (np.arange(NCORES)[:, None, None] * 128 * NC5
                + np.arange(128)[None, :, None] * NC5
                + s + np.arange(NSLOT)[None, None, :]).reshape(-1)
        vf = v.reshape(-1)
        lbase = int(np.cumsum([0] + NS_L[:-1])[l])
        rowid = np.rint(ptab[pidx, 20]).astype(np.int64) - lbase
        core = pidx // (128 * NC5)
        gidx = core * ns + rowid                         # global anchor row
        # tie-aware order: value desc, global index asc (top_k semantics)
        order = np.lexsort((gidx, -vf.astype(np.float64)))[:TOPK]
        vstar = vf[order[-1]]
        if vstar <= NEG / 2:
            return _reference_fallback(inputs)
        # selection safety 1: per-(core,partition) top-8 cut never binding
        percnt = (v >= vstar).sum(axis=2)                # [8, 128]
        if percnt.max() >= NSLOT:
            return _reference_fallback(inputs)
        # selection safety 2: no duplicated ruler value at/above the cut
        # inside any (core,partition) group (max8/max_index tie hazard)
        vs = np.sort(v.reshape(-1, NSLOT), axis=1)
        dup = (vs[:, 1:] == vs[:, :-1]) & (vs[:, 1:] >= vstar) \
            & (vs[:, 1:] > NEG / 2)
        if dup.any():
            return _reference_fallback(inputs)
        if np.unique(gidx[order]).size != TOPK:
            return _reference_fallback(inputs)
        pay = ptab[pidx[order]]
        o = np.empty((TOPK * C, 6), dtype=np.float32)
        o[:, 0:4] = np.repeat(pay[:, 0:4], C, axis=0)
        o[:, 4] = pay[:, 4:20].reshape(-1)
        o[:, 5] = np.tile(np.arange(1, C + 1, dtype=np.float32), TOPK)
        out.append(o)
    return np.concatenate(out, axis=0)
